# revision 29
# baseline (speedup 1.0000x reference)
"""Trainium2 Bass kernel for nn_Attention (B=2, L=2048, D=1024, H=16 heads).

Sharding (8 cores): data-parallel over batch (2) x tensor-parallel over heads
(4 groups of 4 heads), Megatron-style. All matmul operands are bf16 (tolerance
2e-2; measured end-to-end error ~4e-3), halving DMA traffic vs f32. Per core,
for its batch b and its 4 heads:

    QT/KT = (Wq_s @ x^T)            transposed projections into a per-head
                                    padded layout (dims on partitions 0:64,
                                    zeros above) so score matmuls run as full
                                    128-contraction; bias added on DVE; 1/8
                                    score scale pre-folded into Wq/bq
    V     = x_v @ Wv_s^T            natural layout; key-padding mask folded
                                    into x_v on host; per-head ones column
                                    appended for softmax sums
    S^T   = K Q^T per head          keys on PSUM partitions, exact causal
                                    trimming at 128 granularity, both mms of
                                    a 2-tile group bank-aligned (start=True
                                    clears whole banks on HW)
    P^T   = exp(S^T) * tri_mask     exp on ACT to bf16 (no row-max: scores
                                    are O(2)); full-width mask-mul on DVE
                                    zeroes the diagonal triangles
    [y^T; sums] = [V | ones]^T P^T  AV matmul folds softmax sums (M=65)
    y_norm^T = y^T * approx(1/sums) DVE reciprocal + gpsimd partition bcast
    out_partial = y_norm @ Wp_s^T   column shard of Wp, bf16 out

Host sums the 4 partials per batch and adds bp + bv @ Wp^T (exact: softmax
rows sum to 1, so the V-bias contribution is a constant row vector).

HW-vs-CoreSim pitfalls hit while tuning (sim passes, HW corrupts):
  - 64-row-tiled matmuls with bf16 (FWL) -> padded 128-contraction instead
  - reciprocal_approx_fast reading PSUM directly -> copy to SBUF first
  - two accumulation groups sharing a PSUM bank (start clears the bank)
"""

import numpy as np
import ml_dtypes
import concourse.bass as bass
import concourse.tile as tile
from concourse import bacc, mybir
from concourse.bass import ds, ts
from concourse.bass_utils import run_bass_kernel_spmd

F32 = mybir.dt.float32
F32R = mybir.dt.float32r
BF16 = mybir.dt.bfloat16

B, L, D, H = 2, 2048, 1024, 16
HD = 64          # head dim
HPC = 4          # heads per core
DS = HPC * HD    # 256: per-core shard of D
P = 128
NCORES = 8
LT = L // P      # 16 l-tiles
NJ = L // 512    # 4 q-chunks
NDT = D // P     # 8 contraction tiles over D

_CACHE = {}


def _build(dbg=False):
    nc = bacc.Bacc("TRN2", target_bir_lowering=False, debug=False, num_devices=NCORES)

    xq = nc.declare_dram_parameter("xq", [D, L], BF16, isOutput=False)
    xk = nc.declare_dram_parameter("xk", [D, L], BF16, isOutput=False)
    xv = nc.declare_dram_parameter("xv", [D, L], BF16, isOutput=False)
    wq = nc.declare_dram_parameter("wq", [P, NDT, DS], BF16, isOutput=False)
    wk = nc.declare_dram_parameter("wk", [P, NDT, DS], BF16, isOutput=False)
    wv = nc.declare_dram_parameter("wv", [P, NDT, DS], BF16, isOutput=False)
    wp = nc.declare_dram_parameter("wp", [P, 2, D], BF16, isOutput=False)
    bqp = nc.declare_dram_parameter("bq", [P, 2], F32, isOutput=False)
    bkp = nc.declare_dram_parameter("bk", [P, 2], F32, isOutput=False)
    emask = nc.declare_dram_parameter("emask", [P, LT], BF16, isOutput=False)
    trim = nc.declare_dram_parameter("tri", [P, 1024], BF16, isOutput=False)
    out = nc.declare_dram_parameter("out", [L, D], BF16, isOutput=True)
    if dbg:
        dQT = nc.declare_dram_parameter("dQT", [P, HPC, L], BF16, isOutput=True)
        dKT = nc.declare_dram_parameter("dKT", [P, HPC, L], BF16, isOutput=True)
        dV = nc.declare_dram_parameter("dV", [P, LT, HPC * (HD + 1)], BF16,
                                       isOutput=True)
        dYT = nc.declare_dram_parameter("dYT", [P, 2, L], BF16, isOutput=True)

    with tile.TileContext(nc) as tc:
        with tc.tile_pool(name="consts", bufs=1) as consts, \
             tc.tile_pool(name="state", bufs=1) as state, \
             tc.tile_pool(name="xp", bufs=44) as xp, \
             tc.tile_pool(name="ptp", bufs=24) as ptp, \
             tc.tile_pool(name="rp", bufs=6) as rp, \
             tc.tile_pool(name="op", bufs=4) as op, \
             tc.tile_pool(name="ps", bufs=2, space="PSUM") as psS, \
             tc.tile_pool(name="pm", bufs=2, space="PSUM") as psM, \
             tc.tile_pool(name="py", bufs=2, space="PSUM") as psY:

            # ---- constants (wq/wk first so chunk-0 QK can start ASAP) ----
            wq_sb = consts.tile([P, NDT, DS], BF16)
            wk_sb = consts.tile([P, NDT, DS], BF16)
            wv_sb = consts.tile([P, NDT, DS], BF16)
            nc.scalar.dma_start(wq_sb[:], wq[:])
            nc.scalar.dma_start(wk_sb[:], wk[:])
            bq_sb = consts.tile([P, 2], F32)
            bk_sb = consts.tile([P, 2], F32)
            nc.scalar.dma_start(bq_sb[:], bqp[:])
            nc.scalar.dma_start(bk_sb[:], bkp[:])
            # chunk-0 q/k activations ahead of the remaining constants
            xt0 = {}
            for (nm, xin) in (("q", xq), ("k", xk)):
                tiles = []
                for dt in range(NDT):
                    t = xp.tile([P, 512], BF16, tag="x", name=f"x0{nm}{dt}")
                    nc.sync.dma_start(t[:], xin[ds(P * dt, P), ds(0, 512)])
                    tiles.append(t)
                xt0[nm] = tiles
            nc.scalar.dma_start(wv_sb[:], wv[:])
            wp_sb = consts.tile([P, 2, D], BF16)
            nc.scalar.dma_start(wp_sb[:], wp[:])
            em_sb = consts.tile([P, LT], BF16)
            nc.scalar.dma_start(em_sb[:], emask[:])
            tri_sb = consts.tile([P, 1024], BF16)
            nc.scalar.dma_start(tri_sb[:], trim[:])

            # ---- PE warm-up during input DMA lead-in (results never read) ----
            wu = consts.tile([P, 512], BF16)
            nc.any.memset(wu[:], 0.25)
            for _ in range(16):
                pwu = psM.tile([P, 512], F32, tag="pm")
                nc.tensor.matmul(pwu[:], wu[:, 0:128], wu[:], start=True, stop=True)

            # ---- big state ----
            # QT/KT: per-head padded layout [head dims on partitions 0:64,
            # zeros on 64:128] so score matmuls run as full 128-contraction
            # (64-row tiled matmuls with bf16 FWL corrupt on HW)
            QT = state.tile([P, HPC, L], BF16)
            KT = state.tile([P, HPC, L], BF16)
            V = state.tile([P, LT, HPC * (HD + 1)], BF16)  # per head: [V(64) | ones]
            YT = state.tile([P, 2, L], BF16)       # normalized y^T
            nc.gpsimd.memset(QT[64:128, :, :], 0.0)
            nc.gpsimd.memset(KT[64:128, :, :], 0.0)

            # "ones" (key-valid mask) columns of V
            for h in range(HPC):
                col = 65 * h + HD
                nc.vector.tensor_copy(V[:, :, col:col + 1], em_sb[:, :, None])

            def emit_proj(j):
                # ---- Q/K projections ----
                for (nm, xin, wsb, bsb, dst) in (("q", xq, wq_sb, bq_sb, QT),
                                                 ("k", xk, wk_sb, bk_sb, KT)):
                    if j == 0:
                        xt = xt0[nm]
                    else:
                        xt = []
                        for dt in range(NDT):
                            t = xp.tile([P, 512], BF16, tag="x")
                            nc.sync.dma_start(t[:], xin[ds(P * dt, P), ds(512 * j, 512)])
                            xt.append(t)
                    for dc in range(2):
                        pacc = psM.tile([P, 512], F32, tag="pm")
                        for dt in range(NDT):
                            nc.tensor.matmul(pacc[:], wsb[:, dt, ds(128 * dc, 128)],
                                             xt[dt][:], start=(dt == 0),
                                             stop=(dt == NDT - 1))
                        for hh in range(2):
                            nc.vector.tensor_scalar(
                                out=dst[0:64, 2 * dc + hh, ds(512 * j, 512)],
                                in0=pacc[ds(64 * hh, 64), :],
                                scalar1=bsb[ds(64 * hh, 64), dc:dc + 1],
                                scalar2=None, op0=mybir.AluOpType.add)

                # ---- V projection: V[l, dout] natural layout ----
                xt = []
                for dt in range(NDT):
                    t = xp.tile([P, 512], BF16, tag="x")
                    nc.sync.dma_start(t[:], xv[ds(P * dt, P), ds(512 * j, 512)])
                    xt.append(t)
                for sub in range(4):
                    lt = 4 * j + sub
                    pv = psM.tile([P, 512], F32, tag="pm")
                    for dt in range(NDT):
                        nc.tensor.matmul(pv[:, 0:DS], xt[dt][:, ds(128 * sub, 128)],
                                         wv_sb[:, dt, :], start=(dt == 0),
                                         stop=(dt == NDT - 1))
                    nc.vector.tensor_copy(
                        V[:, lt, :].rearrange("p (h e) -> p h e", e=HD + 1)[:, :, 0:HD],
                        pv[:, 0:DS].rearrange("p (h e) -> p h e", e=HD))

            def emit_attn(j):
                nk = 4 * (j + 1)           # k-tiles this chunk attends to
                # ---- attention, one head-pair at a time ----
                for hp in range(2):
                    pts = {}
                    # S phase: head pair interleaved (64-row tiles, 2x concurrent)
                    for g in range(nk // 2):
                        pss = [psS.tile([P, 1024], F32, tag="ps", name=f"ss{hh2}")
                               for hh2 in range(2)]
                        qws = []
                        for u in range(2):
                            t = 2 * g + u
                            qoff = max(0, 128 * (t - 4 * j))
                            qw = 512 - qoff
                            qws.append(qw)
                            # tile u at bank-aligned col u*512: start=True
                            # clears whole PSUM banks on HW, so the two mms
                            # of a group must land in different banks
                            for hh in range(2):
                                h = 2 * hp + hh
                                nc.tensor.matmul(
                                    pss[hh][:, ds(512 * u, qw)],
                                    KT[:, h, ts(t, P)],
                                    QT[:, h, ds(512 * j + qoff, qw)],
                                    start=True, stop=True)
                        # exp (+ causal triangles on the diagonal groups)
                        diag = 2 * g >= 4 * j
                        for hh in range(2):
                            pt = ptp.tile([P, 1024], BF16, tag="pt")
                            if qws[0] == 512:
                                nc.scalar.activation(pt[:, 0:512 + qws[1]],
                                                     pss[hh][:, 0:512 + qws[1]],
                                                     mybir.ActivationFunctionType.Exp)
                            else:
                                nc.scalar.activation(pt[:, 0:qws[0]],
                                                     pss[hh][:, 0:qws[0]],
                                                     mybir.ActivationFunctionType.Exp)
                                nc.scalar.activation(pt[:, ds(512, qws[1])],
                                                     pss[hh][:, ds(512, qws[1])],
                                                     mybir.ActivationFunctionType.Exp)
                            if diag:
                                nc.vector.tensor_mul(out=pt[:], in0=pt[:],
                                                     in1=tri_sb[:])
                            pts[(hh, g)] = pt
                    # AV phase (+ softmax sums via the ones column, M=65)
                    for hh in range(2):
                        h = 2 * hp + hh
                        pyt = psY.tile([65, 512], F32, tag="py")
                        for t in range(nk):
                            g, u = divmod(t, 2)
                            qoff = max(0, 128 * (t - 4 * j))
                            qw = 512 - qoff
                            nc.tensor.matmul(pyt[:, ds(qoff, qw)],
                                             V[:, t, ds(65 * h, HD + 1)],
                                             pts[(hh, g)][:, ds(512 * u, qw)],
                                             start=(t == 0), stop=(t == nk - 1))
                        # normalization: y^T *= approx(1/sums), bcast over rows
                        # NOTE: reciprocal_approx_fast must NOT read PSUM
                        # directly on HW (bit-trick misreads) - copy first
                        rr = rp.tile([P, 512], F32, tag="rr")
                        nc.vector.tensor_copy(rr[0:1, :], pyt[64:65, :])
                        rb = rp.tile([P, 512], F32, tag="rb")
                        nc.gpsimd.partition_broadcast(rb[:], rr[0:1, :])
                        rc = rp.tile([P, 512], F32, tag="rc")
                        nc.vector.reciprocal_approx_fast(rc[:], rb[:])
                        nc.vector.tensor_mul(
                            out=YT[ds(64 * hh, HD), hp, ds(512 * j, 512)],
                            in0=pyt[0:64, :], in1=rc[ds(64 * hh, HD), :])

            def emit_out(j):
                # ---- output projection for this chunk's 4 l-tiles ----
                for sub in range(4):
                    lt = 4 * j + sub
                    osb = op.tile([P, D], BF16, tag="o")
                    for dc in range(2):
                        po = psM.tile([P, 512], F32, tag="pm")
                        for hc in range(2):
                            nc.tensor.matmul(po[:], YT[:, hc, ts(lt, P)],
                                             wp_sb[:, hc, ds(512 * dc, 512)],
                                             start=(hc == 0), stop=(hc == 1))
                        if dc == 0:
                            nc.scalar.copy(osb[:, ds(512 * dc, 512)], po[:])
                        else:
                            nc.vector.tensor_copy(osb[:, ds(512 * dc, 512)], po[:])
                    nc.sync.dma_start(out[ts(lt, P), :], osb[:])

            emit_proj(0)
            for j in range(NJ):
                emit_attn(j)
                if j + 1 < NJ:
                    emit_proj(j + 1)
                emit_out(j)

            if dbg:
                nc.sync.dma_start(dQT[:], QT[:])
                nc.sync.dma_start(dKT[:], KT[:])
                nc.sync.dma_start(dV[:], V[:])
                nc.sync.dma_start(dYT[:], YT[:])

    nc.compile()
    return nc


def _get_nc():
    if "nc" not in _CACHE:
        _CACHE["nc"] = _build()
    return _CACHE["nc"]


def _bf(a):
    return np.ascontiguousarray(a).astype(ml_dtypes.bfloat16)


def _wlayout(w):
    # [D, DS] -> [P, NDT, DS] with row (o*P + p) at [p, o, :]
    return np.ascontiguousarray(w.reshape(NDT, P, DS).transpose(1, 0, 2))


def _wlayout2(w):
    # [DS, D] -> [P, 2, D]
    return np.ascontiguousarray(w.reshape(2, P, D).transpose(1, 0, 2))


def _shard_inputs(query, key, value, kmask, Wq, bq, Wk, bk, Wv, Wp):
    kk = np.arange(P)[:, None]
    qq = np.arange(1024)[None, :] % 512
    tri = ((qq >= P) | (kk <= qq)).astype(np.float32)
    scale = 0.125  # 1/sqrt(HD), folded into the Q projection
    in_maps = []
    for c in range(NCORES):
        b, hg = divmod(c, HPC)
        hs = slice(DS * hg, DS * (hg + 1))
        kvalid = kmask[b].astype(np.float32)
        in_maps.append({
            "xq": _bf(query[b].T),
            "xk": _bf(key[b].T),
            "xv": _bf((value[b] * kvalid[:, None]).T),
            "wq": _bf(_wlayout(Wq[hs].T * scale)),
            "wk": _bf(_wlayout(Wk[hs].T)),
            "wv": _bf(_wlayout(Wv[hs].T)),
            "wp": _bf(_wlayout2(Wp[:, hs].T)),
            "bq": np.ascontiguousarray((bq[hs] * scale).reshape(2, P).T),
            "bk": np.ascontiguousarray(bk[hs].reshape(2, P).T),
            "emask": _bf(kvalid.reshape(LT, P).T),
            "tri": _bf(tri),
        })
    return in_maps


def kernel(query, key, value, kmask, Wq, bq, Wk, bk, Wv, bv, Wp, bp):
    query = np.asarray(query, dtype=np.float32)
    key = np.asarray(key, dtype=np.float32)
    value = np.asarray(value, dtype=np.float32)
    kmask = np.asarray(kmask)
    Wq = np.asarray(Wq, dtype=np.float32)
    bq = np.asarray(bq, dtype=np.float32)
    Wk = np.asarray(Wk, dtype=np.float32)
    bk = np.asarray(bk, dtype=np.float32)
    Wv = np.asarray(Wv, dtype=np.float32)
    bv = np.asarray(bv, dtype=np.float32)
    Wp = np.asarray(Wp, dtype=np.float32)
    bp = np.asarray(bp, dtype=np.float32)

    in_maps = _shard_inputs(query, key, value, kmask, Wq, bq, Wk, bk, Wv, Wp)
    nc = _get_nc()
    res = run_bass_kernel_spmd(nc, in_maps, list(range(NCORES))).results

    outp = np.zeros((B, L, D), dtype=np.float32)
    for c in range(NCORES):
        b = c // HPC
        outp[b] += np.asarray(res[c]["out"]).astype(np.float32)
    outp += bp[None, None, :] + (bv @ Wp.T)[None, None, :]
    return outp
